# revision 81
# baseline (speedup 1.0000x reference)
"""Paged GQA chunked-prefill attention for 8 Trainium2 NeuronCores.

Problem (hardcoded): B=4 seqs x Q=256 new tokens, H=32 query heads, KVH=8 kv
heads (GQA group G=4), D=128 head dim, paged KV cache of 512 blocks x 16
tokens, per-seq lengths in seq_lens (clamped to >= Q), causal masking.

Sharding: tensor-parallel over heads. Core h gets kv head h and query heads
h*4..h*4+3; block_tables/seq_lens are resolved host-side while packing the
shards; the output is all-gathered host-side over the hidden dim.

v3 design (per core; q = (t, g) -> 1024 columns/seq; kv chunks of 128):
  S^T[kv,q] = K_c^T q          fp16 matmul into PSUM (full PE rate)
  u = exp(SCALE*S^T)           ScalarE, PSUM->SBUF, fp16 out
  mask                         multiplicative NEG-add fp16 band tiles via PE
  l_acc += u                   fp16 adds split across VectorE and GpSimd
                               (two parallel chains for the long seq, merged
                               before the l broadcast) -- keeps the
                               denominator reduction OFF the PE
  O^T += V_c^T u               fp16 matmul, PSUM accumulation over chunks
Per-seq: ones-matmul broadcasts the kv-sum of l_acc to all partitions,
reciprocal_approx_fast + multiply on VectorE, DMA out on SP.

Scheduling (tuned against the PE's power-management duty cycling, which
grants ~3.4us-quantized full-speed windows and 50%-duty cooldowns):
longest seq first; PVs trail the QK/exp stream DEEPLY (8 chunks) through
the fill phase so the early cooldown window only has to keep up with
QK-only work, then drain 1/chunk from chunk 10. 6 warm-up matmuls run
during the input-DMA window -- enough continuous PE activity to promote
the clock to 2.4GHz right as real data lands, few enough not to burn the
full-speed grant. Input DMAs: everything rides the SP HWDGE ring in need
order (the ring drains FIFO, so bulk traffic naturally queues behind the
first seq's critical stream and the DMA engines are never shared early);
only the first q half and the small constants ride the ACT ring. The Pool
SWDGE carries no DMA, keeping its exit drain trivial.
Per-seq epilogues run when that seq's PVs leave the trail queue; the
second-to-last seq's lsum+reciprocal run eagerly at its seq end so the
final tail is just multiply + DMA; the last seq's second-half output DMA
goes out on the ACT queue. Fully-masked query columns are skipped (shrunk
matmul/exp widths).
"""
import math

import numpy as np

import concourse.mybir as mybir
import concourse.tile as tile
from concourse import bacc
from concourse.bass_utils import run_bass_kernel_spmd

B, Q, H, D = 4, 256, 32, 128
KVH = 8
G = H // KVH
BLOCK = 16
NB = 128
KV = NB * BLOCK
NUM_BLOCKS = B * NB
SCALE = 1.0 / math.sqrt(D)
N_CORES = 8
CHUNK = 128
QCOLS = G * Q  # 1024 q columns per sequence per core
NHALF = 512

F32 = mybir.dt.float32
F16 = mybir.dt.float16
I16 = mybir.dt.int16

# ---- tuning knobs ----
N_WARM = 6
SHORTEST_FIRST = False
# PV trail: deep during the early chunks (the PE's HAM cooldown window can
# only keep up with QK-only work), draining to shallow once the PE is back
# at full speed.
PV_DEEP = 8
PV_DRAIN_AT = 10
PV_DRAIN_STEP = 1
PV_TRAIL = 2
# For seqs with cb >= POOL_MIN_CB, these chunks' l-adds run on GpSimd as a
# second accumulation chain (merged on VectorE at seq end).
POOL_MIN_CB = 8
POOL_CHAIN = (1, 4, 7, 10)
# Chunks (of pool-eligible seqs) whose exp runs as a Schraudolph bit-trick
# on VectorE instead of ScalarE. Empty = all exp on ScalarE.
SCHR_CHUNKS = ()
# Schraudolph fp16 constants: u = bitcast_fp16(int16(s*SA + SB))
SA = 1024.0 / math.log(2.0) * SCALE
SB = 15.0 * 1024.0 - 0.045 * 1024.0


def _plan(seq_lens):
    """Chunk counts, processing order, per-(seq,chunk,half) mask geometry."""
    L = np.maximum(np.asarray(seq_lens, dtype=np.int64), Q)
    cb = [int((int(x) + CHUNK - 1) // CHUNK) for x in L]
    if SHORTEST_FIRST:
        porder = sorted(range(B), key=lambda b: (cb[b], b))
    else:
        porder = sorted(range(B), key=lambda b: (-cb[b], b))
    offs = {}
    o = 0
    for b in porder:
        offs[b] = o
        o += cb[b]
    C = o
    # info[(b,c,n)]: None if the whole half is masked, else dict with
    # qlo (dead leading cols), blo/bhi (mask band col range within the half)
    info = {}
    for b in range(B):
        Lb = int(L[b])
        for c in range(cb[b]):
            for n in range(2):
                lo = Lb - Q + n * CHUNK  # qpos of this half's first column
                if c * CHUNK > lo + CHUNK - 1:
                    info[(b, c, n)] = None
                    continue
                tdead = min(max(c * CHUNK - lo, 0), CHUNK)
                thi = min(max(c * CHUNK + CHUNK - 1 - lo, 0), CHUNK)
                info[(b, c, n)] = dict(qlo=G * tdead, blo=G * tdead, bhi=G * thi)
    masks = []  # (b, c, n, tdead, thi, moff_t) in processing order
    moff = 0
    for b in porder:
        for c in range(cb[b]):
            for n in range(2):
                st = info[(b, c, n)]
                if st is None or st["bhi"] <= st["blo"]:
                    continue
                td, th = st["blo"] // G, st["bhi"] // G
                masks.append((b, c, n, td, th, moff))
                moff += th - td
    last_n = {
        b: [
            min(cb[b] - 1, (int(L[b]) - Q + n * CHUNK + CHUNK - 1) // CHUNK)
            for n in range(2)
        ]
        for b in range(B)
    }
    return dict(L=L, cb=cb, porder=porder, offs=offs, C=C, info=info,
                masks=masks, mtot=moff, last_n=last_n)


NEG = -20000.0  # exp(SCALE*(s+NEG)) underflows to exactly 0; fp16-exact


def _mask_np(plan):
    m = np.zeros((CHUNK, max(plan["mtot"], 1)), dtype=np.float16)
    p = np.arange(CHUNK)[:, None]
    for (b, c, n, td, th, moff) in plan["masks"]:
        lo = int(plan["L"][b]) - Q + n * CHUNK
        t = np.arange(td, th)[None, :]
        m[:, moff:moff + (th - td)] = np.where(
            c * CHUNK + p <= lo + t, 0.0, NEG
        ).astype(np.float16)
    return m


def _build(seq_lens):
    plan = _plan(seq_lens)
    L, cb, porder, offs = plan["L"], plan["cb"], plan["porder"], plan["offs"]
    C, info, mtot, last_n = plan["C"], plan["info"], plan["mtot"], plan["last_n"]
    midx = {(b, c, n): (td, th, mo)
            for (b, c, n, td, th, mo) in plan["masks"]}
    mask_np = _mask_np(plan)

    nc = bacc.Bacc(
        "TRN2", target_bir_lowering=False, debug=False, num_devices=N_CORES
    )
    kt_d = nc.dram_tensor("kt", [D, C * CHUNK], F16, kind="ExternalInput")
    v_d = nc.dram_tensor("v", [CHUNK, C * CHUNK], F16, kind="ExternalInput")
    qt_d = nc.dram_tensor("qt", [D, B * QCOLS], F16, kind="ExternalInput")
    out_d = nc.dram_tensor("out", [B, D, QCOLS], F16, kind="ExternalOutput")
    mask_d = nc.inline_tensor(mask_np, name="mask_const")
    identb_np = np.eye(CHUNK, dtype=np.float16)
    identb_d = nc.inline_tensor(identb_np, name="identb_const")

    exp = mybir.ActivationFunctionType.Exp
    bf = porder[0]
    qbase = {b: i * QCOLS for i, b in enumerate(porder)}

    def kvcols(b):
        return offs[b] * CHUNK, (offs[b] + cb[b]) * CHUNK

    # per-seq add-chain assignment: which chunks accumulate on GpSimd
    pool_chunks = {
        b: set(c for c in POOL_CHAIN if 0 < c < cb[b] - 1)
        if cb[b] >= POOL_MIN_CB else set()
        for b in range(B)
    }
    schr_chunks = {
        b: set(c for c in SCHR_CHUNKS
               if 0 <= c < cb[b]
               and (b, c, 0) not in midx and (b, c, 1) not in midx
               and info[(b, c, 0)] is not None and info[(b, c, 1)] is not None)
        if cb[b] >= POOL_MIN_CB else set()
        for b in range(B)
    }

    with tile.TileContext(nc) as tc:
        with (
            tc.tile_pool(name="sbin", bufs=1) as sbin,
            tc.tile_pool(name="sbu", bufs=11) as sbu,
            tc.tile_pool(name="sbe", bufs=2) as sbe,
            tc.tile_pool(name="ps_s", bufs=3, space="PSUM") as ps_s,
            tc.tile_pool(name="ps_o", bufs=1, space="PSUM") as ps_o,
        ):
            kt_t = sbin.tile([D, C * CHUNK], F16, tag="kt")
            v_t = sbin.tile([CHUNK, C * CHUNK], F16, tag="v")
            qt_t = sbin.tile([D, B * QCOLS], F16, tag="qt")
            warm = sbin.tile([CHUNK, NHALF], F16, tag="warm")
            masks_t = sbin.tile([CHUNK, max(mtot, 1)], F16, tag="masks")
            lacc = {
                b: sbin.tile([CHUNK, QCOLS], F16, tag=f"lacc{b}", name=f"lacc{b}")
                for b in range(B)
            }
            laccp = {
                b: sbin.tile([CHUNK, QCOLS], F16, tag=f"laccp{b}",
                             name=f"laccp{b}")
                for b in range(B) if pool_chunks[b]
            }
            identb_t = sbin.tile([CHUNK, CHUNK], F16, tag="identb")
            ones_t = sbin.tile([CHUNK, CHUNK], F16, tag="ones")

            nc.vector.memset(warm[:], 0.0)
            nc.vector.memset(ones_t[:], 1.0)

            # ---- input DMAs, first-needed first. SP alone carries the first
            # seq's critical stream (kt/qt/v in chunk order) so the DMA
            # engines aren't shared with bulk traffic early; ACT gets only
            # the tiny constants (its queue must be free for exp); Pool
            # (SWDGE) drains everything else in need order.
            k0, k1 = kvcols(bf)
            qb0 = qbase[bf]
            nc.sync.dma_start(
                kt_t[:, k0:k0 + 2 * CHUNK], kt_d.ap()[:, k0:k0 + 2 * CHUNK]
            )
            # first q half rides the ACT HWDGE ring concurrently with kt
            nc.scalar.dma_start(
                qt_t[:, qb0:qb0 + NHALF], qt_d.ap()[:, qb0:qb0 + NHALF]
            )
            nc.sync.dma_start(
                qt_t[:, qb0 + NHALF:qb0 + QCOLS],
                qt_d.ap()[:, qb0 + NHALF:qb0 + QCOLS],
            )
            nc.sync.dma_start(
                v_t[:, k0:k0 + 2 * CHUNK], v_d.ap()[:, k0:k0 + 2 * CHUNK]
            )
            cut = k0 + 2 * CHUNK
            while cut < k1:
                hi = min(cut + 4 * CHUNK, k1)
                nc.sync.dma_start(kt_t[:, cut:hi], kt_d.ap()[:, cut:hi])
                nc.sync.dma_start(v_t[:, cut:hi], v_d.ap()[:, cut:hi])
                cut = hi
            # ACT queue: tiny constants (exp table load follows them)
            nc.scalar.dma_start(identb_t[:], identb_d.ap())
            if mtot:
                nc.scalar.dma_start(masks_t[:], mask_d.ap())
            # SP after the critical stream: 2nd seq's kt
            s1lo, s1hi = kvcols(porder[1])
            nc.sync.dma_start(kt_t[:, s1lo:s1hi], kt_d.ap()[:, s1lo:s1hi])
            # Bulk traffic rides the SAME SP ring, in need order: the ring
            # drains FIFO, so everything naturally queues behind the first
            # seq's critical stream without any gating, the DMA engines are
            # never shared early, and the Pool SWDGE stays idle (cheap exit
            # drain).
            qs1 = qbase[porder[1]]
            qs2 = min(qbase[porder[2]], qbase[porder[3]])
            nc.sync.dma_start(
                qt_t[:, qs1:qs1 + QCOLS], qt_d.ap()[:, qs1:qs1 + QCOLS]
            )
            nc.sync.dma_start(v_t[:, s1lo:s1hi], v_d.ap()[:, s1lo:s1hi])
            if s1hi < C * CHUNK:
                nc.sync.dma_start(
                    kt_t[:, s1hi:C * CHUNK], kt_d.ap()[:, s1hi:C * CHUNK]
                )
                nc.sync.dma_start(
                    v_t[:, s1hi:C * CHUNK], v_d.ap()[:, s1hi:C * CHUNK]
                )
            nc.sync.dma_start(qt_t[:, qs2:], qt_d.ap()[:, qs2:])

            # ---- PE warm-up (p-state ramp) on zeroed tile ----
            wps = ps_s.tile([CHUNK, QCOLS], F32, tag="s")
            for _ in range(N_WARM):
                nc.tensor.matmul(
                    wps[:, 0:NHALF], warm[:, 0:CHUNK], warm[:],
                    start=True, stop=True,
                )

            def nlive(b, n):
                return sum(
                    1 for c in range(cb[b]) if info[(b, c, n)] is not None
                )

            def chunk_alo(b, c):
                st = [info[(b, c, 0)], info[(b, c, 1)]]
                return st[0]["qlo"] if st[0] is not None else NHALF + st[1]["qlo"]

            def emit_qk(b, c, s_ps):
                st = [info[(b, c, 0)], info[(b, c, 1)]]
                kc = (offs[b] + c) * CHUNK
                qb = qbase[b]
                for n in range(2):
                    if st[n] is None:
                        continue
                    a = n * NHALF + st[n]["qlo"]
                    z = (n + 1) * NHALF
                    mi = midx.get((b, c, n))
                    nc.tensor.matmul(
                        s_ps[:, a:z],
                        kt_t[:, kc:kc + CHUNK],
                        qt_t[:, qb + a:qb + z],
                        start=True, stop=mi is None,
                    )
                    if mi is not None:
                        td, th, mo = mi
                        mb = (
                            masks_t[:, mo:mo + th - td]
                            .unsqueeze(2)
                            .broadcast_to([CHUNK, th - td, G])
                        )
                        nc.tensor.matmul(
                            s_ps[:, n * NHALF + G * td:n * NHALF + G * th],
                            identb_t[:], mb,
                            start=False, stop=True,
                        )

            def emit_exp(b, c, u, s_ps):
                alo = chunk_alo(b, c)
                if c in schr_chunks[b]:
                    nc.vector.tensor_scalar(
                        u[:, alo:QCOLS].bitcast(I16),
                        s_ps[:, alo:QCOLS],
                        SA, SB,
                        mybir.AluOpType.mult, mybir.AluOpType.add,
                    )
                else:
                    nc.scalar.activation(
                        u[:, alo:QCOLS], s_ps[:, alo:QCOLS], exp, scale=SCALE
                    )

            # ---- l accumulation chains (VectorE main, GpSimd side-chain)
            lstate = {b: dict(v=None, p=None, u0=None) for b in range(B)}

            def emit_lacc(b, c, u, skip_l=False):
                if skip_l:
                    return
                stt = lstate[b]
                alo = chunk_alo(b, c)
                if c == 0:
                    stt["u0"] = u
                    return
                if c in pool_chunks[b]:
                    # side chain on GpSimd (alo == 0 chunks only by policy)
                    if stt["p"] is None:
                        stt["p"] = u
                    elif stt["p"] is laccp[b]:
                        nc.gpsimd.tensor_add(
                            laccp[b][:, alo:], laccp[b][:, alo:], u[:, alo:]
                        )
                    else:
                        nc.gpsimd.tensor_add(
                            laccp[b][:], stt["p"][:], u[:]
                        )
                        stt["p"] = laccp[b]
                    return
                st = [info[(b, c, 0)], info[(b, c, 1)]]
                if stt["v"] is None:
                    u0 = stt["u0"]
                    nc.vector.tensor_add(
                        lacc[b][:, alo:], u0[:, alo:], u[:, alo:]
                    )
                    # cols chunk 0 covers but chunk c does not (band edge)
                    for n in range(2):
                        if st[n] is not None and st[n]["qlo"] > 0:
                            gs, ge = n * NHALF, n * NHALF + st[n]["qlo"]
                            nc.scalar.copy(lacc[b][:, gs:ge], u0[:, gs:ge])
                    stt["v"] = lacc[b]
                else:
                    nc.vector.tensor_add(
                        lacc[b][:, alo:], lacc[b][:, alo:], u[:, alo:]
                    )

            def merge_lacc(b):
                stt = lstate[b]
                if stt["p"] is not None:
                    src = stt["p"] if stt["p"] is laccp[b] else None
                    if src is None:
                        # single pool chunk never got a partner; add directly
                        nc.vector.tensor_add(
                            lacc[b][:], lacc[b][:], stt["p"][:]
                        )
                    else:
                        nc.vector.tensor_add(
                            lacc[b][:], lacc[b][:], laccp[b][:]
                        )
                    stt["p"] = None

            def emit_pv(b, c, u, o_ps):
                st = [info[(b, c, 0)], info[(b, c, 1)]]
                kc = (offs[b] + c) * CHUNK
                for n in range(2):
                    if st[n] is None:
                        continue
                    a = n * NHALF + st[n]["qlo"]
                    z = (n + 1) * NHALF
                    nc.tensor.matmul(
                        o_ps[:, a:z],
                        v_t[:, kc:kc + CHUNK],
                        u[:, a:z],
                        start=c == 0, stop=c == last_n[b][n],
                    )

            def emit_lsum_half(b, u0, n, lbc):
                # broadcast column-sum: all-ones lhsT replicates the kv-sum
                # of l_acc into every PSUM partition
                hs = slice(n * NHALF, (n + 1) * NHALF)
                src_t = lacc[b] if nlive(b, n) >= 2 else u0
                nc.tensor.matmul(
                    lbc[:, hs], ones_t[:], src_t[:, hs],
                    start=True, stop=True,
                )

            def emit_epilogue_head(eb, u0):
                # runs once the seq's PVs drain from the trail queue
                lbc = ps_s.tile([CHUNK, QCOLS], F32, tag="s", name="lbc")
                emit_lsum_half(eb, u0, 0, lbc)
                emit_lsum_half(eb, u0, 1, lbc)
                rr = sbe.tile([CHUNK, QCOLS], F32, tag="rl", name="rl")
                nc.vector.reciprocal_approx_fast(rr[:], lbc[:])
                return rr

            def emit_epilogue_tail(eb, eo, rr):
                osb = sbe.tile([D, QCOLS], F16, tag="osb", name="osb")
                nc.vector.tensor_mul(osb[:], eo[:], rr[:])
                nc.sync.dma_start(out_d.ap()[eb][:, :], osb[:])

            # ---- uniform per-seq pipeline, longest seq first.
            # PVs trail the QK/exp stream by PV_TRAIL chunks ACROSS seq
            # boundaries so the PE never drains at a boundary.
            pvq = []  # (b, c, u, o_ps) awaiting emission
            gi_box = [0]  # global chunk counter across seqs

            def push_pv(b, c, u, o_ps):
                pvq.append((b, c, u, o_ps))
                gi = gi_box[0]
                gi_box[0] += 1
                depth = (
                    PV_DEEP if gi < PV_DRAIN_AT
                    else max(PV_TRAIL,
                             PV_DEEP - PV_DRAIN_STEP * (gi - PV_DRAIN_AT))
                )
                while len(pvq) > depth:
                    emit_pv(*pvq.pop(0))

            def flush_pv():
                while pvq:
                    emit_pv(*pvq.pop(0))

            def flush_seq_pv(sb):
                while pvq and pvq[0][0] == sb:
                    emit_pv(*pvq.pop(0))

            pends = []  # (b, o_ps, rr) awaiting their PVs to drain
            for bi, b in enumerate(porder):
                lastseq = bi == len(porder) - 1
                shortlast = lastseq and cb[b] <= 3
                o_ps = ps_o.tile([D, QCOLS], F32, tag="o", name="o")
                u0 = None
                lbc_l = None
                rl_l = None
                osb_l = None
                for c in range(cb[b]):
                    uu = sbu.tile([CHUNK, QCOLS], F16, tag="u", name="u")
                    s_ps = ps_s.tile([CHUNK, QCOLS], F32, tag="s", name="s")
                    emit_qk(b, c, s_ps)
                    emit_exp(b, c, uu, s_ps)
                    emit_lacc(b, c, uu, skip_l=shortlast)
                    if shortlast:
                        # accumulate l straight in PSUM: ones-matmul per
                        # live half of this chunk; reciprocal per half as
                        # soon as that half's accumulation stops
                        if lbc_l is None:
                            lbc_l = ps_s.tile(
                                [CHUNK, QCOLS], F32, tag="s", name="lbc_l"
                            )
                            rl_l = sbe.tile(
                                [CHUNK, QCOLS], F32, tag="rl", name="rl"
                            )
                        for n in range(2):
                            if info[(b, c, n)] is None:
                                continue
                            hs = slice(n * NHALF, (n + 1) * NHALF)
                            nc.tensor.matmul(
                                lbc_l[:, hs], ones_t[:], uu[:, hs],
                                start=c == 0, stop=c == last_n[b][n],
                            )
                            if c == last_n[b][n]:
                                nc.vector.reciprocal_approx_fast(
                                    rl_l[:, hs], lbc_l[:, hs]
                                )
                    push_pv(b, c, uu, o_ps)
                    # previous seqs' outputs: only once all of their PVs
                    # have left the trail queue (o_ps accumulation is done)
                    while pends and all(
                        e[0] != pends[0][0] for e in pvq
                    ):
                        pb, po, pay, isrr = pends.pop(0)
                        prr = pay if isrr else emit_epilogue_head(pb, pay)
                        emit_epilogue_tail(pb, po, prr)
                    if c == 0:
                        u0 = uu
                merge_lacc(b)
                if lastseq:
                    if not shortlast:
                        lbc_l = ps_s.tile(
                            [CHUNK, QCOLS], F32, tag="s", name="lbc_l"
                        )
                        emit_lsum_half(b, u0, 0, lbc_l)
                        emit_lsum_half(b, u0, 1, lbc_l)
                        rl_l = sbe.tile(
                            [CHUNK, QCOLS], F32, tag="rl", name="rl"
                        )
                        nc.vector.reciprocal_approx_fast(rl_l[:], lbc_l[:])
                    while pends:
                        pb, po, pay, isrr = pends.pop(0)
                        flush_seq_pv(pb)
                        prr = pay if isrr else emit_epilogue_head(pb, pay)
                        emit_epilogue_tail(pb, po, prr)
                    flush_pv()
                    osb_l = sbe.tile([D, QCOLS], F16, tag="osb", name="osb")
                    nc.vector.tensor_mul(
                        osb_l[:, 0:NHALF], o_ps[:, 0:NHALF], rl_l[:, 0:NHALF]
                    )
                    nc.sync.dma_start(
                        out_d.ap()[b][:, 0:NHALF], osb_l[:, 0:NHALF]
                    )
                    nc.vector.tensor_mul(
                        osb_l[:, NHALF:], o_ps[:, NHALF:], rl_l[:, NHALF:]
                    )
                    nc.scalar.dma_start(
                        out_d.ap()[b][:, NHALF:], osb_l[:, NHALF:]
                    )
                else:
                    if bi == len(porder) - 2:
                        # second-to-last seq: eager head (lsum+recip) so the
                        # final tail is just multiply + DMA
                        rr = emit_epilogue_head(b, u0)
                        pends.append((b, o_ps, rr, True))
                    else:
                        pends.append((b, o_ps, u0, False))
    nc.compile()
    return nc, plan


def _pack_inputs(query, k_cache, v_cache, block_tables, plan):
    """Gather the paged cache and pack per-core fp16 shards."""
    L, cb, porder, offs, C = (
        plan["L"], plan["cb"], plan["porder"], plan["offs"], plan["C"]
    )
    k_lin = k_cache[block_tables].reshape(B, KV, KVH, D)
    v_lin = v_cache[block_tables].reshape(B, KV, KVH, D)
    kt_all = np.zeros((KVH, D, C * CHUNK), dtype=np.float16)
    v_all = np.zeros((KVH, CHUNK, C * CHUNK), dtype=np.float16)
    for b in range(B):
        Lb, w = int(L[b]), cb[b] * CHUNK
        o0 = offs[b] * CHUNK
        kk = np.zeros((w, KVH, D), dtype=np.float32)
        kk[:Lb] = k_lin[b, :Lb]
        kt_all[:, :, o0:o0 + w] = kk.transpose(1, 2, 0).astype(np.float16)
        vv = np.zeros((w, KVH, D), dtype=np.float32)
        vv[:Lb] = v_lin[b, :Lb]
        v_all[:, :, o0:o0 + w] = (
            vv.reshape(cb[b], CHUNK, KVH, D)
            .transpose(2, 1, 0, 3)
            .reshape(KVH, CHUNK, w)
            .astype(np.float16)
        )
    # query [B,Q,H,D] -> porder-major [KVH, D, B*QCOLS] (t-major, g inner)
    qp = query[np.array(porder)]
    qt_all = (
        qp.transpose(2, 3, 0, 1)
        .reshape(KVH, G, D, B, Q)
        .transpose(0, 2, 3, 4, 1)
        .reshape(KVH, D, B * QCOLS)
        .astype(np.float16)
    )
    return [
        {
            "kt": np.ascontiguousarray(kt_all[h]),
            "v": np.ascontiguousarray(v_all[h]),
            "qt": np.ascontiguousarray(qt_all[h]),
        }
        for h in range(KVH)
    ]


def _unpack_outputs(results):
    """[B,D,QCOLS] per core (O^T, q=(t,g) on cols) -> [B*Q, H*D]."""
    out = np.empty((B * Q, H * D), dtype=np.float32)
    for h, res in enumerate(results):
        o = res["out"].reshape(B, D, Q, G)  # [b, d, t, g]
        o = o.transpose(0, 2, 3, 1).reshape(B * Q, G * D)
        out[:, h * G * D:(h + 1) * G * D] = o
    return out


def kernel(query, k_cache, v_cache, block_tables, seq_lens):
    query = np.asarray(query, dtype=np.float32)
    k_cache = np.asarray(k_cache, dtype=np.float32)
    v_cache = np.asarray(v_cache, dtype=np.float32)
    block_tables = np.asarray(block_tables, dtype=np.int64)
    nc, plan = _build(np.asarray(seq_lens))
    in_maps = _pack_inputs(query, k_cache, v_cache, block_tables, plan)
    res = run_bass_kernel_spmd(nc, in_maps, core_ids=list(range(N_CORES)))
    return _unpack_outputs(res.results)
